# revision 1
# baseline (speedup 1.0000x reference)
"""Trainium2 Bass kernel for nn_LossRegressionGaussianWithCorrelations.

Loss = (1/50) * (lp_var - lp_prior) - lp_lik with
  lp_prior = sum(-0.5*noisy^2 - 0.5*log2pi) / 512
  lp_lik   = sum(-0.5*((mu_pred - y)/sigma)^2 - log(sigma) - 0.5*log2pi) / 512
  lp_var   = sum_s(-0.5*(1024*log2pi + logdet(Sigma) + maha_s)) / 512,
             maha_s = dx_s^T Sigma^-1 dx_s,  dx = noisy - mu_w

Distribution (8 cores):
  - mu_prediction [65536,512] / y_true sharded along batch (8192 rows/core),
    streamed at the SBUF-port DMA ceiling (~430 GB/s/core). The likelihood
    sum is expanded as sum(mu^2) - 2*sum_b y_b r_b + 512*sum(y^2) so that
    sum(mu^2) runs as one big grouped Square per DMA block (alternating
    ScalarE/VectorE, accum_out row-sums) and the cross term runs on the
    otherwise-idle TensorE as M=1 f32r matmuls y^T @ mu_tile accumulated
    into a single PSUM bank.
  - maha: tr(Sinv @ dx^T dx) sharded over 128-column slices of Sinv/G per
    core; G columns via TensorE bf16 matmuls on dx (replicated noisy),
    multiply+reduce with VectorE scalar_tensor_tensor accum.
  - prior: replicated (VectorE mult+reduce), core 0's result used.
  - Host does the O(n^3) inherently sequential part in fp64 (Cholesky ->
    logdet, inv) and the final fp64 combine of fp32 partials.
"""

import ml_dtypes
import numpy as np

BF16 = ml_dtypes.bfloat16

N_CORES = 8
P = 128          # partitions
BATCH = 65536
S = 512          # n_samples
W = 1024         # n_weights
RPC = BATCH // N_CORES   # batch rows per core = 8192
NT = RPC // P    # lik tiles per core (batch rows per partition) = 64
# lik DMA block sizes (in 256KB tiles); small first/last blocks cut ramp/lag
BLOCKS = [2, 4, 6, 8, 8, 8, 8, 8, 8, 4]
NB = len(BLOCKS)
WA = S // P      # s-chunks of noisy = 4
NI = W // P      # i-chunks of G rows = 8
JC = W // N_CORES  # G/Sinv columns per core = 128

_STATE = {}


def _build_program():
    import concourse.bacc as bacc
    import concourse.bass as bass
    import concourse.mybir as mybir
    from concourse import tile

    f32 = mybir.dt.float32
    f32r = mybir.dt.float32r
    bf16 = mybir.dt.bfloat16
    nc = bacc.Bacc("TRN2", num_devices=N_CORES)

    # mu/y are f32r (same bits as fp32) so the y^T@mu cross-term matmuls run
    # single-pass on the PE; ACT/DVE consumers bitcast back to f32.
    mu = nc.dram_tensor("mu", [RPC, S], f32r, kind="ExternalInput").ap()
    y = nc.dram_tensor("y", [P, NT], f32r, kind="ExternalInput").ap()
    noisy = nc.dram_tensor("noisy", [S, W], bf16, kind="ExternalInput").ap()
    muw = nc.dram_tensor("muw", [1, W], bf16, kind="ExternalInput").ap()
    muwc = nc.dram_tensor("muwc", [1, JC], bf16, kind="ExternalInput").ap()
    ncol = nc.dram_tensor("ncol", [S, JC], bf16, kind="ExternalInput").ap()
    sinv = nc.dram_tensor("sinv", [W, JC], bf16, kind="ExternalInput").ap()

    out_acc = nc.dram_tensor("out_acc", [P, NB + 1 + WA + NI], f32,
                             kind="ExternalOutput").ap()
    out_ym = nc.dram_tensor("out_ym", [1, S], f32, kind="ExternalOutput").ap()

    # batch row b = p*NT + t lives at partition p, tile t
    mu_v = mu.rearrange("(p t) s -> p t s", p=P)         # [128, 64, 512]
    noisy_v = noisy.rearrange("(a p) w -> p a w", p=P)   # [128, 4, 1024]
    ncol_v = ncol.rearrange("(a p) j -> p a j", p=P)     # [128, 4, 128]
    sinv_v = sinv.rearrange("(a p) j -> p a j", p=P)     # [128, 8, 128]

    def bcast(ap_1xn, n):
        # DRAM [1, n] read with partition step 0 -> broadcast to 128 partitions
        return bass.AP(tensor=ap_1xn.tensor, offset=ap_1xn.offset,
                       ap=[[0, P], [1, n]])

    with tile.TileContext(nc) as tc:
        with (
            tc.tile_pool(name="const", bufs=1) as const,
            tc.tile_pool(name="mup", bufs=4) as mup,
            tc.tile_pool(name="dump", bufs=1) as dumps,
            tc.tile_pool(name="gpsum", bufs=2, space="PSUM") as gpsum,
            tc.tile_pool(name="scr", bufs=2) as scr,
        ):
            y_sb = const.tile([P, NT], f32r)
            nc.sync.dma_start(out=y_sb, in_=y)

            acc = const.tile([P, NB + 1 + WA + NI], f32)
            acc_mu2 = acc[:, 0:NB]
            acc_y2 = acc[:, NB:NB + 1]
            acc_pri = acc[:, NB + 1:NB + 1 + WA]
            acc_maha = acc[:, NB + 1 + WA:]
            zero_b = const.tile([P, 1], f32)
            nc.vector.memset(zero_b, 0.0)

            # ---- likelihood stream ----
            # sum(mu - y)^2 = sum(mu^2) - 2*sum_b y_b r_b + 512*sum(y^2):
            #  * sum(mu^2): one grouped Square per DMA block, alternating
            #    ScalarE ACT / VectorE STT (both with accum row-sums);
            #  * sum_b y_b r_b: per 512-col tile, an M=1 TensorE matmul
            #    y_col^T @ mu_tile accumulated into one [1,512] PSUM bank;
            #  * sum(y^2): one tiny STT on y_sb.
            ym = gpsum.tile([1, S], f32, tag="ym")
            n_mm = [0]

            def lik_block(bi, j0, tb, eng):
                mt = mup.tile([P, tb, S], f32r, tag="mt")
                eng.dma_start(out=mt, in_=mu_v[:, j0:j0 + tb, :])
                flat = mt.rearrange("p t s -> p (t s)").bitcast(f32)
                if bi % 2 == 0 or bi == 1:
                    dummy = dumps.tile([P, tb * S], f32, tag="dummy")
                    nc.scalar.activation(
                        out=dummy, in_=flat,
                        func=mybir.ActivationFunctionType.Square,
                        scale=1.0, bias=zero_b,
                        accum_out=acc_mu2[:, bi:bi + 1])
                else:
                    sq = scr.tile([P, tb * S], f32, tag="sq")
                    nc.vector.scalar_tensor_tensor(
                        out=sq, in0=flat, scalar=1.0, in1=flat,
                        op0=mybir.AluOpType.mult, op1=mybir.AluOpType.mult,
                        accum_out=acc_mu2[:, bi:bi + 1])
                for t in range(tb):
                    k = j0 + t
                    nc.tensor.matmul(
                        out=ym, lhsT=y_sb[:, k:k + 1], rhs=mt[:, t, :],
                        start=(n_mm[0] == 0), stop=(n_mm[0] == NT - 1),
                        skip_group_check=True)
                    n_mm[0] += 1

            lik_block(0, 0, BLOCKS[0], nc.sync)

            noisy_sb = const.tile([P, WA, W], bf16)
            nc.sync.dma_start(out=noisy_sb, in_=noisy_v)
            ncol_sb = const.tile([P, WA, JC], bf16)
            nc.sync.dma_start(out=ncol_sb, in_=ncol_v)
            sinv_sb = const.tile([P, NI, JC], bf16)
            nc.sync.dma_start(out=sinv_sb, in_=sinv_v)
            muw_sb = const.tile([P, W], bf16)
            nc.gpsimd.dma_start(out=muw_sb, in_=bcast(muw, W))
            muwc_sb = const.tile([P, JC], bf16)
            nc.gpsimd.dma_start(out=muwc_sb, in_=bcast(muwc, JC))

            j0 = BLOCKS[0]
            for bi, tb in enumerate(BLOCKS[1:]):
                # alternate the two HWDGE rings (SP via nc.sync, ACT via
                # nc.scalar) so each ring sees half the stream
                lik_block(bi + 1, j0, tb, nc.scalar if bi % 2 == 0 else nc.sync)
                j0 += tb
            assert j0 == NT
            y2scr = scr.tile([P, NT], f32, tag="y2scr")
            nc.vector.scalar_tensor_tensor(
                out=y2scr, in0=y_sb.bitcast(f32), scalar=1.0,
                in1=y_sb.bitcast(f32), op0=mybir.AluOpType.mult,
                op1=mybir.AluOpType.mult, accum_out=acc_y2)
            ym_sb = const.tile([1, S], f32)
            nc.vector.tensor_copy(out=ym_sb, in_=ym)
            nc.sync.dma_start(out=out_ym, in_=ym_sb)

            # ---- prior (sum noisy^2) and dx = noisy - mu_w ----
            dx_sb = const.tile([P, WA, W], bf16)
            dxc_sb = const.tile([P, WA, JC], bf16)
            for a in range(WA):
                pscr = scr.tile([P, W], f32, tag="pscr")
                # out = (noisy * 1.0) * noisy, accum_out = row-sum(out)
                # (tensor_tensor_reduce faults on this HW path; STT works).
                nc.vector.scalar_tensor_tensor(
                    out=pscr, in0=noisy_sb[:, a, :], scalar=1.0,
                    in1=noisy_sb[:, a, :],
                    op0=mybir.AluOpType.mult, op1=mybir.AluOpType.mult,
                    accum_out=acc_pri[:, a:a + 1])
                nc.vector.tensor_sub(
                    out=dx_sb[:, a, :], in0=noisy_sb[:, a, :], in1=muw_sb)
                nc.gpsimd.tensor_sub(
                    out=dxc_sb[:, a, :], in0=ncol_sb[:, a, :], in1=muwc_sb)

            # ---- G column block + maha partials ----
            for i in range(NI):
                g = gpsum.tile([P, JC], f32, tag="g")
                for a in range(WA):
                    nc.tensor.matmul(
                        out=g, lhsT=dx_sb[:, a, i * P:(i + 1) * P],
                        rhs=dxc_sb[:, a, :],
                        start=(a == 0), stop=(a == WA - 1))
                gscr = scr.tile([P, JC], f32, tag="gscr")
                nc.vector.scalar_tensor_tensor(
                    out=gscr, in0=g, scalar=1.0, in1=sinv_sb[:, i, :],
                    op0=mybir.AluOpType.mult, op1=mybir.AluOpType.mult,
                    accum_out=acc_maha[:, i:i + 1])

            nc.sync.dma_start(out=out_acc, in_=acc)

    nc.compile()
    return nc


def _get_nc():
    if "nc" not in _STATE:
        _STATE["nc"] = _build_program()
    return _STATE["nc"]


def kernel(**inputs):
    noisy = np.ascontiguousarray(np.asarray(inputs["noisy_weights"], dtype=np.float32))
    mu_w = np.ascontiguousarray(np.asarray(inputs["mu_weights"], dtype=np.float32))
    Sigma = np.asarray(inputs["sigma_matrix_weights"])
    mu_p = np.ascontiguousarray(np.asarray(inputs["mu_prediction"], dtype=np.float32))
    sig_p = float(np.asarray(inputs["sigma_prediction"]))
    y = np.ascontiguousarray(np.asarray(inputs["y_true"], dtype=np.float32))

    # Host: the O(n^3) inherently-sequential factorization, in float64.
    S64 = Sigma.astype(np.float64)
    try:
        L = np.linalg.cholesky(S64)
    except np.linalg.LinAlgError:
        # jnp.linalg.cholesky yields NaNs for a non-SPD matrix, which
        # propagate to a NaN loss in the reference — match that.
        return np.float32(np.nan)
    logdet = 2.0 * float(np.sum(np.log(np.diagonal(L))))
    Sinv32 = np.linalg.inv(S64).astype(np.float32)

    nc = _get_nc()
    noisy16 = noisy.astype(BF16)
    muw16 = mu_w.astype(BF16)
    sinv16 = Sinv32.astype(BF16)
    in_maps = []
    for c in range(N_CORES):
        in_maps.append({
            "mu": mu_p[c * RPC:(c + 1) * RPC],
            "y": y[c * RPC:(c + 1) * RPC].reshape(P, NT),
            "noisy": noisy16,
            "muw": muw16.reshape(1, W),
            "muwc": np.ascontiguousarray(muw16[c * JC:(c + 1) * JC]).reshape(1, JC),
            "ncol": np.ascontiguousarray(noisy16[:, c * JC:(c + 1) * JC]),
            "sinv": np.ascontiguousarray(sinv16[:, c * JC:(c + 1) * JC]),
        })

    from concourse.bass_utils import run_bass_kernel_spmd
    res = run_bass_kernel_spmd(nc, in_maps, core_ids=list(range(N_CORES)))

    NBv = len(BLOCKS)
    S_mu2 = float(sum(res.results[c]["out_acc"][:, 0:NBv].astype(np.float64).sum()
                      for c in range(N_CORES)))
    S_y2 = float(sum(res.results[c]["out_acc"][:, NBv:NBv + 1].astype(np.float64).sum()
                     for c in range(N_CORES)))
    S_yr = float(sum(res.results[c]["out_ym"].astype(np.float64).sum()
                     for c in range(N_CORES)))
    S_lik = S_mu2 - 2.0 * S_yr + S * S_y2
    S_pri = float(res.results[0]["out_acc"][:, NBv + 1:NBv + 1 + WA]
                  .astype(np.float64).sum())
    S_maha = float(sum(
        res.results[c]["out_acc"][:, NBv + 1 + WA:].astype(np.float64).sum()
        for c in range(N_CORES)))

    log2pi = float(np.log(2.0 * np.pi))
    lp_prior = (-0.5 * S_pri - 0.5 * log2pi * (S * W)) / S
    lp_lik = (-0.5 * S_lik / (sig_p * sig_p)
              - (np.log(sig_p) + 0.5 * log2pi) * (BATCH * S)) / S
    lp_var = -0.5 * (S * W * log2pi + S * logdet + S_maha) / S
    total = (lp_var - lp_prior) / 50.0 - lp_lik
    return np.float32(total)

